# revision 24
# baseline (speedup 1.0000x reference)
"""MoE layer (8 routed experts, top-2, shared experts) on 8 Trainium2 cores.

Strategy: expert parallelism. Core c owns routed expert c (dense compute over
all tokens, weighted by that expert's combine column) plus a 1/8 shard of the
shared expert (MS split). Each core produces a full [N, H] partial sum; a
chunked ReduceScatter over the token dim combines them, so core r ends with
tokens {c*256 + r*32 .. +32} for each 256-token chunk c. The host reassembles.

Routing (gate logits, top-2, softmax) is computed on-device in exact fp32;
expert/shared matmuls run as float32r (fast fp32 PE mode).
"""

import sys

if "/opt/trn_rl_repo" not in sys.path:
    sys.path.insert(0, "/opt/trn_rl_repo")

import numpy as np

# ---- problem constants (hardcoded per contest contract) ----
B, S, H = 2, 1024, 2048
N = B * S                # 2048 tokens
E = 8                    # routed experts = number of cores
M = 512                  # moe intermediate
MS = 1024                # shared intermediate total
MS_SH = MS // E          # 128 per core
P = 128
KT = H // P              # 16 contraction tiles
MT = M // P              # 4 routed m-tiles
HC = 4                   # output H chunks of 512
HALF = N // 2            # 1024 tokens per half
NSH = HALF // P          # 8 token slices per half
NCORES = 8

_CACHE = {}


def _build_program(collectives=True, loop_n=None):
    import concourse.bass as bass
    import concourse.mybir as mybir
    import concourse.tile as tile
    from concourse import bacc
    from concourse.masks import make_identity
    from contextlib import ExitStack

    f32 = mybir.dt.float32
    f32r = mybir.dt.float32r

    nc = bacc.Bacc(None)

    x_d = nc.declare_dram_parameter("x", [N, H], f32, isOutput=False)
    gwt_d = nc.declare_dram_parameter("gwt", [P, KT * E], f32, isOutput=False)
    wg_d = nc.declare_dram_parameter("wg", [MT, P, KT * P], f32r, isOutput=False)
    wu_d = nc.declare_dram_parameter("wu", [MT, P, KT * P], f32r, isOutput=False)
    wd_d = nc.declare_dram_parameter("wd", [M, H], f32r, isOutput=False)
    swg_d = nc.declare_dram_parameter("swg", [P, KT * P], f32r, isOutput=False)
    swu_d = nc.declare_dram_parameter("swu", [P, KT * P], f32r, isOutput=False)
    swd_d = nc.declare_dram_parameter("swd", [MS_SH, H], f32r, isOutput=False)
    sel_d = nc.declare_dram_parameter("sel", [P, E], f32, isOutput=False)
    out_d = nc.declare_dram_parameter("out", [N // NCORES, H], f32, isOutput=True)

    rg = [list(range(NCORES))]

    with tile.TileContext(nc) as tc:
        with (
            tc.tile_pool(name="sb", bufs=1) as sb,
            tc.tile_pool(name="stream", bufs=6) as wpool,
            tc.tile_pool(name="small", bufs=2) as sm,
            tc.tile_pool(name="ps_a", bufs=4, space="PSUM") as ps_a,
            tc.tile_pool(name="ps_b", bufs=4, space="PSUM") as ps_b,
            tc.tile_pool(name="dram", bufs=1, space="DRAM") as dram,
        ):
            y_part = [
                dram.tile([N // 8, H], f32, name=f"ypart{c}", tag=f"ypart{c}")
                for c in range(8)
            ]
            y_rs = [
                dram.tile([N // 64, H], f32, name=f"yrs{c}", tag=f"yrs{c}")
                for c in range(8)
            ]

            ident = sb.tile([P, P], f32, name="ident")
            make_identity(nc, ident[:])
            gwt_t = sb.tile([P, KT, E], f32, name="gwt_t")
            nc.sync.dma_start(gwt_t[:], gwt_d[:].rearrange("p (kt e) -> p kt e", e=E))
            sel_t = sb.tile([P, E], f32, name="sel_t")
            nc.sync.dma_start(sel_t[:], sel_d[:])
            combw = sb.tile([P, N // P], f32, name="combw")

            # persistent per-chunk buffers (2 chunks of 512 tokens per half)
            xTr_ch = [
                sb.tile([P, KT, 512], f32r, name=f"xTr{c}") for c in range(2)
            ]
            aT_ch = [sb.tile([P, MT, 512], f32r, name=f"aT{c}") for c in range(2)]
            asT_ch = [sb.tile([P, 512], f32r, name=f"asT{c}") for c in range(2)]

            loop_ctx = ExitStack()
            if loop_n is not None:
                loop_ctx.enter_context(tc.For_i(0, loop_n, 1))

            for half in range(2):
                tok0 = half * HALF

                # ---------- Phase A: transpose x + exact fp32 gate logits ----------
                lga = sm.tile([P, NSH, E], f32, name=f"lga_{half}", tag="lga")
                t8a = sm.tile([P, NSH, 8], f32, name=f"t8a_{half}", tag="t8a")
                for ns in range(NSH):
                    nt = half * NSH + ns
                    ch, cns = ns // 4, ns % 4
                    x_in = wpool.tile([P, H], f32, name=f"x_{nt}", tag="x_in", bufs=2)
                    nc.sync.dma_start(
                        x_in[:], x_d[tok0 + ns * P : tok0 + (ns + 1) * P, :]
                    )
                    xTf32 = wpool.tile(
                        [P, KT, P], f32, name=f"xT32_{nt}", tag="xTf32", bufs=2
                    )
                    for g4 in range(4):
                        psA = ps_a.tile([P, 512], f32, name=f"psA_{nt}_{g4}", tag="pa")
                        for j in range(4):
                            kt = g4 * 4 + j
                            nc.tensor.transpose(
                                psA[:, j * P : (j + 1) * P],
                                x_in[:, kt * P : (kt + 1) * P],
                                ident[:],
                            )
                        ps3 = psA[:].rearrange("p (j c) -> p j c", j=4)
                        xtr_dst = xTr_ch[ch][
                            :, g4 * 4 : (g4 + 1) * 4, cns * P : (cns + 1) * P
                        ]
                        xtf_dst = xTf32[:, g4 * 4 : (g4 + 1) * 4, :]
                        if g4 % 2 == 0:
                            nc.scalar.copy(xtr_dst, ps3)
                            nc.vector.tensor_copy(xtf_dst, ps3)
                        else:
                            nc.vector.tensor_copy(xtr_dst, ps3)
                            nc.scalar.copy(xtf_dst, ps3)

                    psL = ps_b.tile([P, E], f32, name=f"psL_{nt}", tag="pb")
                    for kt in range(KT):
                        nc.tensor.matmul(
                            psL[:],
                            xTf32[:, kt, :],
                            gwt_t[:, kt, :],
                            start=(kt == 0),
                            stop=(kt == KT - 1),
                        )
                    nc.vector.tensor_copy(lga[:, ns], psL[:])
                    nc.vector.max(t8a[:, ns], lga[:, ns])

                # batched routing math for the half (top-2 softmax combine col)
                AL = mybir.AluOpType
                dm = sm.tile([P, NSH], f32, name=f"dm_{half}", tag="rt1")
                nc.vector.tensor_tensor(
                    dm[:], t8a[:, :, 1], t8a[:, :, 0], AL.subtract
                )
                ew = sm.tile([P, NSH], f32, name=f"ew_{half}", tag="rt2")
                nc.scalar.activation(ew[:], dm[:], mybir.ActivationFunctionType.Exp)
                z = sm.tile([P, NSH], f32, name=f"z_{half}", tag="rt3")
                nc.vector.tensor_scalar_add(z[:], ew[:], 1.0)
                w1 = sm.tile([P, NSH], f32, name=f"w1_{half}", tag="rt4")
                nc.vector.reciprocal(w1[:], z[:])
                w2 = sm.tile([P, NSH], f32, name=f"w2_{half}", tag="rt5")
                nc.vector.tensor_mul(w2[:], ew[:], w1[:])
                mk1 = sm.tile([P, NSH, E], f32, name=f"mk1_{half}", tag="rt6")
                nc.vector.tensor_tensor(
                    mk1[:], lga[:], t8a[:, :, 0:1].to_broadcast([P, NSH, E]),
                    AL.is_equal,
                )
                l2 = sm.tile([P, NSH, E], f32, name=f"l2_{half}", tag="rt7")
                nc.vector.scalar_tensor_tensor(
                    l2[:], mk1[:], -1.0e30, lga[:], AL.mult, AL.add
                )
                mk2 = sm.tile([P, NSH, E], f32, name=f"mk2_{half}", tag="rt8")
                nc.vector.tensor_tensor(
                    mk2[:], l2[:], t8a[:, :, 1:2].to_broadcast([P, NSH, E]),
                    AL.is_equal,
                )
                nc.vector.tensor_tensor(
                    mk1[:], mk1[:], w1[:, :, None].to_broadcast([P, NSH, E]), AL.mult
                )
                nc.vector.tensor_tensor(
                    mk2[:], mk2[:], w2[:, :, None].to_broadcast([P, NSH, E]), AL.mult
                )
                nc.vector.tensor_add(mk1[:], mk1[:], mk2[:])
                nc.vector.tensor_tensor(
                    mk1[:], mk1[:], sel_t[:, None, :].to_broadcast([P, NSH, E]),
                    AL.mult,
                )
                nc.vector.reduce_sum(
                    combw[:, half * NSH : (half + 1) * NSH],
                    mk1[:],
                    axis=mybir.AxisListType.X,
                )

                # ---------- Phase C1: expert gate/up + SwiGLU ----------
                for mt in range(MT):
                    wg_t = wpool.tile(
                        [P, KT, P], f32r, name=f"wg_{half}_{mt}", tag="wst", bufs=4
                    )
                    nc.sync.dma_start(
                        wg_t[:], wg_d[mt].rearrange("p (kt m) -> p kt m", m=P)
                    )
                    wu_t = wpool.tile(
                        [P, KT, P], f32r, name=f"wu_{half}_{mt}", tag="wst", bufs=4
                    )
                    nc.sync.dma_start(
                        wu_t[:], wu_d[mt].rearrange("p (kt m) -> p kt m", m=P)
                    )
                    for ch in range(2):
                        c0 = ch * 512
                        psG = ps_b.tile(
                            [P, 512], f32, name=f"psG_{half}_{mt}_{ch}", tag="pb"
                        )
                        for kt in range(KT):
                            nc.tensor.matmul(
                                psG[:],
                                wg_t[:, kt, :],
                                xTr_ch[ch][:, kt, :],
                                start=(kt == 0),
                                stop=(kt == KT - 1),
                            )
                        psU = ps_b.tile(
                            [P, 512], f32, name=f"psU_{half}_{mt}_{ch}", tag="pb"
                        )
                        for kt in range(KT):
                            nc.tensor.matmul(
                                psU[:],
                                wu_t[:, kt, :],
                                xTr_ch[ch][:, kt, :],
                                start=(kt == 0),
                                stop=(kt == KT - 1),
                            )
                        sil = sm.tile(
                            [P, 512], f32, name=f"sil_{half}_{mt}_{ch}", tag="sil"
                        )
                        nc.scalar.activation(
                            sil[:], psG[:], mybir.ActivationFunctionType.Silu
                        )
                        nc.vector.tensor_mul(aT_ch[ch][:, mt, :], sil[:], psU[:])

                # shared expert shard gate/up
                swg_t = wpool.tile([P, KT, P], f32r, name=f"swg_{half}", tag="wst", bufs=4)
                nc.sync.dma_start(
                    swg_t[:], swg_d[:].rearrange("p (kt m) -> p kt m", m=P)
                )
                swu_t = wpool.tile([P, KT, P], f32r, name=f"swu_{half}", tag="wst", bufs=4)
                nc.sync.dma_start(
                    swu_t[:], swu_d[:].rearrange("p (kt m) -> p kt m", m=P)
                )
                for ch in range(2):
                    c0 = ch * 512
                    psGs = ps_b.tile([P, 512], f32, name=f"psGs_{half}_{ch}", tag="pb")
                    for kt in range(KT):
                        nc.tensor.matmul(
                            psGs[:],
                            swg_t[:, kt, :],
                            xTr_ch[ch][:, kt, :],
                            start=(kt == 0),
                            stop=(kt == KT - 1),
                        )
                    psUs = ps_b.tile([P, 512], f32, name=f"psUs_{half}_{ch}", tag="pb")
                    for kt in range(KT):
                        nc.tensor.matmul(
                            psUs[:],
                            swu_t[:, kt, :],
                            xTr_ch[ch][:, kt, :],
                            start=(kt == 0),
                            stop=(kt == KT - 1),
                        )
                    sils = sm.tile([P, 512], f32, name=f"sils_{half}_{ch}", tag="sil")
                    nc.scalar.activation(
                        sils[:], psGs[:], mybir.ActivationFunctionType.Silu
                    )
                    nc.vector.tensor_mul(asT_ch[ch][:], sils[:], psUs[:])



                # ---------- Phase C2: down proj + combine scale + shared add ----------
                wd_t = wpool.tile([P, MT, H], f32r, name=f"wd_{half}", tag="wd", bufs=1)
                nc.sync.dma_start(
                    wd_t[:], wd_d[:].rearrange("(mt p) h -> p mt h", p=P)
                )
                swd_t = wpool.tile([P, H], f32r, name=f"swd_{half}", tag="swd", bufs=1)
                nc.sync.dma_start(swd_t[:], swd_d[:])
                for ns in range(NSH):
                    nt = half * NSH + ns
                    for hc in range(HC):
                        h0 = hc * 512
                        psY = ps_b.tile(
                            [P, 512], f32, name=f"psY_{nt}_{hc}", tag="pb"
                        )
                        for mt in range(MT):
                            nc.tensor.matmul(
                                psY[:],
                                aT_ch[ns // 4][:, mt, (ns % 4) * P : (ns % 4 + 1) * P],
                                wd_t[:, mt, h0 : h0 + 512],
                                start=(mt == 0),
                                stop=(mt == MT - 1),
                            )
                        psS = ps_b.tile(
                            [P, 512], f32, name=f"psS_{nt}_{hc}", tag="pb"
                        )
                        nc.tensor.matmul(
                            psS[:],
                            asT_ch[ns // 4][:, (ns % 4) * P : (ns % 4 + 1) * P],
                            swd_t[:, h0 : h0 + 512],
                            start=True,
                            stop=True,
                        )
                        yt = sm.tile([P, 512], f32, name=f"yt_{nt}_{hc}", tag="yt", bufs=3)
                        nc.scalar.activation(
                            yt[:],
                            psY[:],
                            mybir.ActivationFunctionType.Copy,
                            scale=combw[:, nt : nt + 1],
                        )
                        nc.vector.tensor_add(yt[:], yt[:], psS[:])
                        cchunk = (nt * P) // 256
                        crow = (nt * P) % 256
                        nc.sync.dma_start(
                            y_part[cchunk][crow : crow + P, h0 : h0 + 512], yt[:]
                        )
                    # token chunk complete after its odd slice: fire its RS
                    if ns % 2 == 1:
                        c = (half * NSH + ns) // 2
                        if collectives:
                            nc.gpsimd.collective_compute(
                                "ReduceScatter",
                                mybir.AluOpType.add,
                                replica_groups=rg,
                                ins=[y_part[c][:]],
                                outs=[y_rs[c][:]],
                            )
                            nc.sync.dma_start(
                                out_d[c * 32 : (c + 1) * 32, :], y_rs[c][:]
                            )
                        else:
                            nc.sync.dma_start(
                                out_d[c * 32 : (c + 1) * 32, :], y_part[c][:32, :]
                            )

            loop_ctx.close()

    nc.finalize()
    return nc


def _prep_in_maps(inputs) -> list:
    x = np.ascontiguousarray(
        np.asarray(inputs["hidden_states"], dtype=np.float32).reshape(N, H)
    )
    gate_w = np.asarray(inputs["gate_w"], dtype=np.float32)
    Wg = np.asarray(inputs["Wg"], dtype=np.float32)
    Wu = np.asarray(inputs["Wu"], dtype=np.float32)
    Wd = np.asarray(inputs["Wd"], dtype=np.float32)
    sWg = np.asarray(inputs["sWg"], dtype=np.float32)
    sWu = np.asarray(inputs["sWu"], dtype=np.float32)
    sWd = np.asarray(inputs["sWd"], dtype=np.float32)

    # device-friendly layouts: partition-major tiles so every weight DMA is
    # a fully contiguous transfer
    def tile_km(w):  # [H, Mw] -> [P, KT*Mw] with [p, kt, m] = w[kt*P+p, m]
        mw = w.shape[1]
        return np.ascontiguousarray(
            w.reshape(KT, P, mw).transpose(1, 0, 2).reshape(P, KT * mw)
        )

    def tile_km_mt(w):  # [H, M] -> [MT, P, KT*P] split by m-tile
        return np.ascontiguousarray(
            w.reshape(KT, P, MT, P).transpose(2, 1, 0, 3).reshape(MT, P, KT * P)
        )

    gwt = tile_km(np.ascontiguousarray(gate_w.T))  # [P, KT*E]

    in_maps = []
    for c in range(NCORES):
        sel = np.zeros((P, E), dtype=np.float32)
        sel[:, c] = 1.0
        in_maps.append(
            {
                "x": x,
                "gwt": gwt,
                "wg": tile_km_mt(Wg[c]),
                "wu": tile_km_mt(Wu[c]),
                "wd": np.ascontiguousarray(Wd[c]),
                "swg": tile_km(sWg[:, c * MS_SH : (c + 1) * MS_SH]),
                "swu": tile_km(sWu[:, c * MS_SH : (c + 1) * MS_SH]),
                "swd": np.ascontiguousarray(sWd[c * MS_SH : (c + 1) * MS_SH, :]),
                "sel": sel,
            }
        )
    return in_maps


def _unshard(results) -> np.ndarray:
    # core r's output rows are tokens c*256 + r*32 .. +32 for chunk c in 0..7
    y = np.empty((N, H), dtype=np.float32)
    for r in range(NCORES):
        o = results[r]["out"]  # [256, H]
        for c in range(8):
            y[c * 256 + r * 32 : c * 256 + (r + 1) * 32] = o[c * 32 : (c + 1) * 32]
    return y.reshape(B, S, H)


def kernel(**inputs) -> np.ndarray:
    from concourse.bass_utils import run_bass_kernel_spmd

    in_maps = _prep_in_maps(inputs)

    if "nc" not in _CACHE:
        _CACHE["nc"] = _build_program()
    nc = _CACHE["nc"]

    res = run_bass_kernel_spmd(nc, in_maps, list(range(NCORES))).results
    return _unshard(res)


if __name__ == "__main__":
    # smoke test against the local reference
    sys.path.insert(0, "/root/problem")
    import reference

    inp = reference.setup_inputs()
    expected = np.asarray(reference.reference(**inp))
    actual = kernel(**{k: np.asarray(v) for k, v in inp.items()})
    err = np.linalg.norm(actual - expected) / np.linalg.norm(expected)
    print("Relative error:", err)
